# revision 25
# baseline (speedup 1.0000x reference)
"""BERT-with-RoPE attention layer on 8 Trainium2 NeuronCores.

Sharding: core c handles (batch b = c//2, head-half hh = c%2): 8 of the 16
heads over the FULL 2048-token sequence. q/k/v are computed only for the
core's own heads (no duplicated work); Wout is row-sharded (vLLM style), so
each core emits a full-shape PARTIAL output and the all-reduce degenerates
to a host-side sum of the two partials per batch (free - not on device).

Single fused device pipeline per core:
  The attention stream (scores -> exp -> ctx) is ACT-engine bound (~1us per
  key-block for the 1024-elem exp).  All other matmul work - q/k projection
  chains with RoPE, v projection groups, out-projection groups - is emitted
  as PE "filler" between attention blocks so the tensor engine computes
  underneath the exp stream instead of in separate phases.  This also keeps
  the PE HAM clock-gate warm (no idle windows > 3.4us).

  PSUM budget (8 banks): scores double-buffered (2 tiles x 2 banks), ctx
  accumulators cE/cO (2 banks, single-buffered - freed fast via DVE copy to
  SBUF right after the last ctx matmul), shared 2-bank spare pool for all
  filler accumulations (projection chains, rope-swap, v, out-proj).

  Matmuls in bf16 (fp32 PSUM accumulate); softmax in fp32 via ACT exp with
  fused 1/8 scale; NeoX RoPE halves-swap as a float32r PE permutation
  matmul; softmax sums via a ones column appended to v (65th PSUM row).
"""

import os
import numpy as np
from collections import deque

B, S, H = 4, 2048, 1024
NH, DH = 16, 64
HALF = DH // 2
KC = H // 128        # x contraction chunks
NOC = 8              # q/k output col chunks per core (0-3 q, 4-7 k)
PRS = 4              # head pairs per core
NQS = 4              # 512-col query spans
KCD = 4              # out-proj contraction chunks (512 rows / 128)
ROPE_BASE = 10000.0
N_CORES = 8

_nc_cache = None
last_results = None


def _build_nc():
    import concourse.bacc as bacc
    import concourse.mybir as mybir
    from concourse.tile import TileContext

    f32 = mybir.dt.float32
    f32r = mybir.dt.float32r
    bf16 = mybir.dt.bfloat16
    Exp = mybir.ActivationFunctionType.Exp
    MUL = mybir.AluOpType.mult
    ADD = mybir.AluOpType.add

    nc = bacc.Bacc(None, target_bir_lowering=False)

    xT_d = nc.dram_tensor("xT", [128, KC, S], bf16, kind="ExternalInput")
    wqk_d = nc.dram_tensor("wqk", [NOC, 128, KC, 128], bf16, kind="ExternalInput")
    wv_d = nc.dram_tensor("wv", [128, KC, 512], bf16, kind="ExternalInput")
    wout_d = nc.dram_tensor("wout", [128, 8, KCD, 128], bf16, kind="ExternalInput")
    pswap_d = nc.dram_tensor("pswap", [128, 128], f32r, kind="ExternalInput")
    cosk_d = nc.dram_tensor("cosk", [128, S], f32, kind="ExternalInput")
    sink_d = nc.dram_tensor("sink", [128, S], f32, kind="ExternalInput")
    bqk_d = nc.dram_tensor("bqk", [128, NOC], f32, kind="ExternalInput")
    out_d = nc.dram_tensor("outT", [8, 128, S], bf16, kind="ExternalOutput")
    debug = bool(int(os.environ.get("KERNEL_DEBUG", "0") or "0"))
    if debug:
        dq_d = nc.dram_tensor("dq", [128, PRS, S], bf16, kind="ExternalOutput")
        dk_d = nc.dram_tensor("dk", [128, PRS, S], bf16, kind="ExternalOutput")
        dv_d = nc.dram_tensor("dv", [128, 16, 8, DH + 1], bf16, kind="ExternalOutput")
        dctx_d = nc.dram_tensor("dctx", [128, PRS, S], bf16, kind="ExternalOutput")

    with TileContext(nc) as tc:
        with (
            tc.tile_pool(name="const", bufs=1) as const,
            tc.tile_pool(name="persist", bufs=1) as persist,
            tc.tile_pool(name="wqkp", bufs=1) as wqkp,
            tc.tile_pool(name="ropep", bufs=2) as ropep,
            tc.tile_pool(name="expp", bufs=3) as expp,
            tc.tile_pool(name="scrp", bufs=2) as scrp,
            tc.tile_pool(name="obp", bufs=2) as obp,
            tc.tile_pool(name="spare", bufs=2, space="PSUM") as spare,
            tc.tile_pool(name="psSc", bufs=2, space="PSUM") as psSc,
            tc.tile_pool(name="psCtx", bufs=1, space="PSUM") as psCtx,
        ):
            pswap_sb = const.tile([128, 128], f32r)
            nc.sync.dma_start(pswap_sb[:, :], pswap_d[:, :])
            bqk_sb = const.tile([128, NOC], f32)
            nc.sync.dma_start(bqk_sb[:, :], bqk_d[:, :])

            xT_sb = persist.tile([128, KC, S], bf16)
            cosk_sb = persist.tile([128, S], f32)
            sink_sb = persist.tile([128, S], f32)
            qTr = persist.tile([128, PRS, S], bf16)
            kTr = persist.tile([128, PRS, S], bf16)
            v_sb = persist.tile([128, 16, 8, DH + 1], bf16)
            ctxT = persist.tile([128, PRS, S], bf16)
            wv_sb = persist.tile([128, KC, 512], bf16)
            wout_sb = persist.tile([128, 8, KCD, 128], bf16)

            # ones column of v via memset - scattered 2-byte DMAs would
            # serialize ~10us of Sync-engine issue time at the queue head
            nc.gpsimd.memset(v_sb[:, :, :, DH:DH + 1], 1.0)

            # ---------------- DMA emission (single FIFO queue: order by
            # consumption; each dma_start costs ~0.65us of Sync issue, so
            # few big DMAs).  k(p0)+q(p0) weights, then xT quarter-by-
            # quarter with sin/cos, wv after the first quarter. ---------
            wqk_tiles = {}

            def load_wqk(oc):
                # per-oc tag (bufs=1): a reused slot would make the weight
                # DMA wait on a free-semaphore, blocking the Sync FIFO
                if oc not in wqk_tiles:
                    w = wqkp.tile([128, KC, 128], bf16, tag=f"w{oc}",
                                  name=f"wqk{oc}")
                    nc.sync.dma_start(w[:, :, :], wqk_d[oc, :, :, :])
                    wqk_tiles[oc] = w
                return wqk_tiles[oc]

            def load_gen(oc):
                load_wqk(oc)
                yield

            load_wqk(4)  # k chunk 0 (pair 0)
            load_wqk(0)  # q chunk 0 (pair 0)
            for qs in range(NQS):
                sl = slice(qs * 512, (qs + 1) * 512)
                nc.sync.dma_start(xT_sb[:, :, sl], xT_d[:, :, sl])
                nc.sync.dma_start(sink_sb[:, sl], sink_d[:, sl])
                nc.sync.dma_start(cosk_sb[:, sl], cosk_d[:, sl])
                if qs == 0:
                    nc.sync.dma_start(wv_sb[:, :, :], wv_d[:, :, :])

            # ---------------- building blocks ----------------
            # q/k chains are split in two stages, software-pipelined one
            # chain deep: stage 2 of chain N is emitted after stage 1 of
            # chain N+1 so its rope-swap matmul (which waits on DVE work)
            # never blocks the in-order PE queue.
            chain_state = {}

            def chain_s1(oc, qs):
                w = load_wqk(oc)
                sl = slice(qs * 512, (qs + 1) * 512)
                ps = spare.tile([128, 512], f32, tag="ps", name="psqk")
                for c in range(4):
                    nc.tensor.matmul(ps[:, :], w[:, c, :], xT_sb[:, c, sl],
                                     start=(c == 0), stop=False)
                yield
                for c in range(4, KC):
                    nc.tensor.matmul(ps[:, :], w[:, c, :], xT_sb[:, c, sl],
                                     start=False, stop=(c == KC - 1))
                raw = ropep.tile([128, 512], f32, tag="raw", name="raw")
                nc.vector.tensor_scalar_add(raw[:, :], ps[:, :],
                                            bqk_sb[:, oc:oc + 1])
                tt = ropep.tile([128, 512], f32r, tag="tt", name="tt")
                nc.vector.tensor_tensor(tt[:, :], raw[:, :], sink_sb[:, sl], MUL)
                chain_state[(oc, qs)] = (raw, tt)
                yield

            def chain_s2(oc, qs):
                raw, tt = chain_state.pop((oc, qs))
                sl = slice(qs * 512, (qs + 1) * 512)
                sw = spare.tile([128, 512], f32, tag="ps", name="pssw")
                nc.tensor.matmul(sw[:, :], pswap_sb[:, :], tt[:, :],
                                 start=True, stop=True)
                cc = ropep.tile([128, 512], f32, tag="cc", name="cc")
                nc.gpsimd.tensor_tensor(cc[:, :], raw[:, :], cosk_sb[:, sl], MUL)
                dst = qTr[:, oc, sl] if oc < 4 else kTr[:, oc - 4, sl]
                nc.vector.tensor_tensor(dst, cc[:, :], sw[:, :], ADD)
                yield

            def vgroup_p0(sb):
                """v projection for head pair 0 (128 cols) of seq block sb."""
                sbc = slice(sb * 128, (sb + 1) * 128)
                ps = spare.tile([128, 512], f32, tag="ps", name="psv0")
                for c in range(KC):
                    nc.tensor.matmul(
                        ps[:, 0:128], xT_sb[:, c, sbc], wv_sb[:, c, 0:128],
                        start=(c == 0), stop=(c == KC - 1),
                    )
                nc.vector.tensor_copy(
                    v_sb[:, sb, 0:2, 0:DH],
                    ps[:, 0:128].rearrange("p (h d) -> p h d", h=2),
                )

            def vrest_gen(sb, lo, hi, hlo):
                """v projection for head cols [lo:hi) of seq block sb."""
                sbc = slice(sb * 128, (sb + 1) * 128)
                w = hi - lo
                nh = w // DH
                ps = spare.tile([128, 512], f32, tag="ps", name="psvr")
                for c in range(KC):
                    nc.tensor.matmul(
                        ps[:, 0:w], xT_sb[:, c, sbc], wv_sb[:, c, lo:hi],
                        start=(c == 0), stop=(c == KC - 1),
                    )
                    if c == 3:
                        yield
                nc.vector.tensor_copy(
                    v_sb[:, sb, hlo:hlo + nh, 0:DH],
                    ps[:, 0:w].rearrange("p (h d) -> p h d", h=nh),
                )
                yield

            def wout_load_gen():
                nc.sync.dma_start(wout_sb[:, :, :, :], wout_d[:, :, :, :])
                yield

            dg_state = {}

            def dgroup_s1(oc, qs):
                sl = slice(qs * 512, (qs + 1) * 512)
                ps = spare.tile([128, 512], f32, tag="ps", name="pso")
                for c in range(KCD):
                    nc.tensor.matmul(
                        ps[:, :], wout_sb[:, oc, c, :], ctxT[:, c, sl],
                        start=(c == 0), stop=(c == KCD - 1),
                    )
                dg_state[(oc, qs)] = ps
                yield

            def dgroup_s2(oc, qs):
                ps = dg_state.pop((oc, qs))
                sl = slice(qs * 512, (qs + 1) * 512)
                ob = obp.tile([128, 512], bf16, tag="ob", name="ob")
                nc.vector.tensor_copy(ob[:, :], ps[:, :])
                nc.sync.dma_start(out_d[oc, :, sl], ob[:, :])
                yield

            # ---------------- filler schedule ----------------
            # (ready, deadline, generator), queue roughly deadline-sorted.
            # pump() advances the head between attention blocks but never
            # starts an item before `ready` (so a far-future dep can't park
            # in the in-order engine queues).  drain_due() force-emits every
            # item whose deadline has arrived BEFORE the consuming iteration
            # is emitted - program order is semantic order in Tile, so a
            # consumer emitted before its producer would read garbage.
            # Chains appear as interleaved (s1 of next, s2 of previous)
            # pairs; vrest items are sprinkled between pairs.
            chain_order = (
                [(0, qs, 0, qs) for qs in range(1, NQS)]          # q(p0)
                + [(0, 4, 5, qs) for qs in range(NQS)]            # k(p1)
                + [(max(1, qs), 4 + qs, 1, qs) for qs in range(NQS)]  # q(p1)
                + [(4, 8, 6, qs) for qs in range(NQS)]            # k(p2)
                + [(4, 8 + qs, 2, qs) for qs in range(NQS)]       # q(p2)
                + [(8, 12, 7, qs) for qs in range(NQS)]           # k(p3)
                + [(8, 12 + qs, 3, qs) for qs in range(NQS)]      # q(p3)
            )
            # v projections for pairs 1-3, two waves: pair 1 (128 cols) due
            # iter 4, pairs 2-3 (256 cols) due iter 8
            vrest_items = deque(
                [(1, 4, vrest_gen(sb, 128, 256, 2)) for sb in range(16)]
                + [(4, 8, vrest_gen(sb, 256, 512, 4)) for sb in range(16)])
            fillers = deque()
            fillers.append((0, 3, load_gen(5)))
            fillers.append((0, 3, load_gen(1)))
            prev = None
            for (r, dl, oc, qs) in chain_order:
                if (oc, qs) == (6, 0):
                    fillers.append((2, 7, load_gen(6)))
                    fillers.append((2, 7, load_gen(2)))
                if (oc, qs) == (7, 0):
                    fillers.append((5, 11, load_gen(7)))
                    fillers.append((5, 11, load_gen(3)))
                fillers.append((r, dl, chain_s1(oc, qs)))
                if prev is not None:
                    pr_, pdl_, poc, pqs = prev
                    fillers.append((pr_, pdl_, chain_s2(poc, pqs)))
                prev = (r, dl, oc, qs)
                nv = 0
                while vrest_items and vrest_items[0][1] <= dl + 1 and nv < 2:
                    fillers.append(vrest_items.popleft())
                    nv += 1
            pr_, pdl_, poc, pqs = prev
            fillers.append((pr_, pdl_, chain_s2(poc, pqs)))
            while vrest_items:
                fillers.append(vrest_items.popleft())
            fillers.append((8, 13, wout_load_gen()))
            dprev = None
            for qs in range(NQS):             # out-proj after C(p3, sp=qs)
                for oc in range(8):
                    fillers.append((13 + qs, 99, dgroup_s1(oc, qs)))
                    if dprev is not None:
                        fillers.append((13 + qs, 99, dgroup_s2(*dprev)))
                    dprev = (oc, qs)
            fillers.append((16, 99, dgroup_s2(*dprev)))

            cur_iter = [0]

            def pump(n):
                while n > 0 and fillers:
                    ready, _, gen = fillers[0]
                    if ready > cur_iter[0]:
                        return
                    try:
                        next(gen)
                        n -= 1
                    except StopIteration:
                        fillers.popleft()

            def drain_due(it):
                # run-to-completion every queued item whose deadline has
                # arrived, even behind later-deadline items (same-chain s1
                # precedes s2 in queue order, so intra-chain order holds)
                keep = []
                for (r, dl, gen) in fillers:
                    if dl <= it:
                        for _ in gen:
                            pass
                    else:
                        keep.append((r, dl, gen))
                fillers.clear()
                fillers.extend(keep)

            # ---------------- preamble: k(p0) span 0 + q(p0) span 0; the
            # remaining k(p0) spans are emitted inline in iteration 0 right
            # before the score blocks that consume them, pacing with the
            # xT DMA stream instead of serializing the full load up front.
            for _ in chain_s1(4, 0):
                pass
            for _ in chain_s1(0, 0):
                pass
            for _ in chain_s2(4, 0):
                pass
            for _ in chain_s2(0, 0):
                pass
            iter0_q = deque()
            for qs in range(1, NQS):
                iter0_q.append((4 * qs - 1, chain_s1(4, qs)))
                iter0_q.append((4 * qs - 1, chain_s2(4, qs)))

            def pump0(blk):
                # drain everything due before this block, else one step
                while iter0_q and iter0_q[0][0] < blk:
                    for _ in iter0_q.popleft()[1]:
                        pass
                if iter0_q:
                    try:
                        next(iter0_q[0][1])
                    except StopIteration:
                        iter0_q.popleft()

            # ---------------- fused attention stream ----------------
            for pr in range(PRS):
                for sp in range(NQS):
                    it = pr * NQS + sp
                    cur_iter[0] = it
                    drain_due(it)
                    s1 = slice(sp * 512, (sp + 1) * 512)
                    cE = psCtx.tile([128, 512], f32, tag="ce", name="cE")
                    cO = psCtx.tile([128, 512], f32, tag="co", name="cO")
                    for blk in range(16):
                        if it == 0:
                            pump0(blk)
                        sc = psSc.tile([128, 2, 512], f32, tag="sc", name="sc")
                        for par in range(2):
                            rs = par * 64
                            nc.tensor.matmul(
                                sc[:, par, :],
                                kTr[rs:rs + 64, pr, blk * 128:(blk + 1) * 128],
                                qTr[rs:rs + 64, pr, s1],
                                start=True, stop=True,
                            )
                        et = expp.tile([128, 2, 512], bf16, tag="et", name="et")
                        nc.scalar.activation(et[:, :, :], sc[:, :, :], Exp,
                                             scale=0.125)
                        if it == 0:
                            vgroup_p0(blk)
                        else:
                            pump(1)
                        st, sp_ = (blk == 0), (blk == 15)
                        nc.tensor.matmul(
                            cE[0:DH + 1, :], v_sb[:, blk, 2 * pr, :],
                            et[:, 0, :], start=st, stop=sp_,
                        )
                        nc.tensor.matmul(
                            cO[0:DH + 1, :], v_sb[:, blk, 2 * pr + 1, :],
                            et[:, 1, :], start=st, stop=sp_,
                        )
                    # epilogue part 1: drain ctx accumulators to SBUF fast
                    # so the single-buffered PSUM banks free immediately.
                    ctf = scrp.tile([128, 2, 512], f32, tag="ctf", name="ctf")
                    nc.vector.tensor_copy(ctf[0:DH + 1, 0, :], cE[0:DH + 1, :])
                    nc.vector.tensor_copy(ctf[0:DH + 1, 1, :], cO[0:DH + 1, :])
                    # sums row (partition 64) -> partition 0 via DMA: the
                    # broadcast/recip ops act on tensor partition 0 only.
                    scr2 = scrp.tile([1, 2, 512], f32, tag="scr2", name="scr2")
                    nc.sync.dma_start(scr2[0:1, :, :], ctf[DH:DH + 1, :, :])
                    pump(3)
                    # epilogue part 2: normalize off the critical path.
                    bcs = scrp.tile([64, 2, 512], f32, tag="bcs", name="bcs")
                    nc.gpsimd.partition_broadcast(bcs[0:DH, :, :],
                                                  scr2[0:1, :, :])
                    bc = scrp.tile([64, 2, 512], f32, tag="bc", name="bc")
                    nc.vector.reciprocal_approx_fast(bc[0:DH, :, :],
                                                     bcs[0:DH, :, :])
                    nc.vector.tensor_tensor(ctxT[0:DH, pr, s1],
                                            ctf[0:DH, 0, :], bc[0:DH, 0, :],
                                            MUL)
                    tmp = scrp.tile([64, 512], bf16, tag="tmp", name="tmp")
                    nc.vector.tensor_tensor(tmp[:, :], ctf[0:DH, 1, :],
                                            bc[0:DH, 1, :], MUL)
                    nc.sync.dma_start(ctxT[DH:128, pr, s1], tmp[:, :])
                    pump(3)

            # ---------------- drain remaining fillers ----------------
            cur_iter[0] = 99
            while fillers:
                pump(100)

            if debug:
                nc.sync.dma_start(dq_d[:, :, :], qTr[:, :, :])
                nc.sync.dma_start(dk_d[:, :, :], kTr[:, :, :])
                nc.sync.dma_start(dv_d[:, :, :, :], v_sb[:, :, :, :])
                nc.sync.dma_start(dctx_d[:, :, :], ctxT[:, :, :])

    nc.finalize()
    return nc


def _host_prep(positions, hidden_states, Wqkv, bqkv, Wout, bout):
    import ml_dtypes

    bf16 = ml_dtypes.bfloat16
    hidden_states = np.asarray(hidden_states, dtype=np.float32)
    Wqkv = np.asarray(Wqkv, dtype=np.float32)
    bqkv = np.asarray(bqkv, dtype=np.float32)
    Wout = np.asarray(Wout, dtype=np.float32)
    positions = np.asarray(positions)

    pswap = np.zeros((128, 128), dtype=np.float32)
    for m in range(128):
        if m % DH < HALF:
            pswap[m + HALF, m] = -1.0
        else:
            pswap[m - HALF, m] = 1.0

    inv_freq = 1.0 / (ROPE_BASE ** (np.arange(HALF, dtype=np.float64) / HALF))
    rowmap = np.arange(128) % HALF
    freqs = positions.astype(np.float64)[:, None] * inv_freq[None, :]  # [S, 32]
    cosk = np.ascontiguousarray(np.cos(freqs).astype(np.float32)[:, rowmap].T)
    sink = np.ascontiguousarray(np.sin(freqs).astype(np.float32)[:, rowmap].T)

    xTs = []
    for b in range(B):
        # [128, KC, S]: partition-major so one DMA covers all chunks
        xTs.append(np.ascontiguousarray(
            hidden_states[b].T.reshape(KC, 128, S).transpose(1, 0, 2)
        ).astype(bf16))

    per_hh = []
    for hh in range(2):
        qsl = slice(hh * 512, (hh + 1) * 512)
        ksl = slice(H + hh * 512, H + (hh + 1) * 512)
        vsl = slice(2 * H + hh * 512, 2 * H + (hh + 1) * 512)
        wq = np.ascontiguousarray(
            Wqkv[:, qsl].reshape(KC, 128, 4, 128).transpose(2, 1, 0, 3))
        wk = np.ascontiguousarray(
            Wqkv[:, ksl].reshape(KC, 128, 4, 128).transpose(2, 1, 0, 3))
        wqk = np.concatenate([wq, wk], axis=0).astype(bf16)  # [8,128,KC,128]
        wv = np.ascontiguousarray(
            Wqkv[:, vsl].reshape(KC, 128, 512).transpose(1, 0, 2)).astype(bf16)
        wout = np.ascontiguousarray(
            Wout[hh * 512:(hh + 1) * 512, :]
            .reshape(KCD, 128, 8, 128).transpose(1, 2, 0, 3)).astype(bf16)
        bq = bqkv[:H][qsl].reshape(4, 128).T
        bk = bqkv[H:2 * H][hh * 512:(hh + 1) * 512].reshape(4, 128).T
        bqk = np.ascontiguousarray(np.concatenate([bq, bk], axis=1))  # [128,8]
        per_hh.append((wqk, wv, wout, bqk))

    in_maps = []
    for c in range(N_CORES):
        b, hh = c // 2, c % 2
        wqk, wv, wout, bqk = per_hh[hh]
        in_maps.append({
            "xT": xTs[b], "wqk": wqk, "wv": wv, "wout": wout,
            "pswap": pswap, "cosk": cosk, "sink": sink,
            "bqk": bqk,
        })
    return in_maps


def kernel(positions, hidden_states, Wqkv, bqkv, Wout, bout):
    global _nc_cache, last_results
    from concourse import bass_utils

    if _nc_cache is None:
        _nc_cache = _build_nc()
    nc = _nc_cache

    in_maps = _host_prep(positions, hidden_states, Wqkv, bqkv, Wout, bout)
    res = bass_utils.run_bass_kernel_spmd(
        nc, in_maps, core_ids=list(range(N_CORES)),
        trace=bool(int(os.environ.get("KERNEL_TRACE", "0") or "0")),
    )
    last_results = res

    bqkv = np.asarray(bqkv, dtype=np.float32)
    Wout = np.asarray(Wout, dtype=np.float32)
    bout = np.asarray(bout, dtype=np.float32)
    # v-bias contribution (attn rows sum to 1) + output bias, added on host
    bias_full = (bout + bqkv[2 * H:].astype(np.float64) @
                 Wout.astype(np.float64)).astype(np.float32)

    out = np.empty((B, S, H), dtype=np.float32)
    for b in range(B):
        p0 = np.asarray(res.results[2 * b]["outT"]).astype(np.float32)
        p1 = np.asarray(res.results[2 * b + 1]["outT"]).astype(np.float32)
        o = (p0 + p1).reshape(H, S)
        out[b] = o.T + bias_full[None, :]
    return out
